# revision 6
# baseline (speedup 1.0000x reference)
"""KNN top-5 kernel for Trainium2 (Bass/Tile), SPMD over 8 NeuronCores.

Problem: x [16384, 256] f32, reference_points [100, 256] f32.
Output: indices [16384, 5] int32 of the 5 nearest reference points per row
(ascending distance, ties -> lower index), matching
jax.lax.top_k(-||x - r||, 5).

Strategy (v3):
  - Data parallel: 2048 rows of x per core; reference table replicated.
  - Ranking by v = 2*x.r - ||r||^2 (per-row monotone in -d), accumulated in
    fp32 PSUM.  fp32 matmuls are AVOIDED: on TRN2 each fp32 matmul lowers to
    LOW+HIGH passes whose LDWEIGHTS cannot be pulled ahead (HW hang guard)
    and which disable FWL -> ~213 ns/pass.  Instead x and q=2r^T are split
    host-side into bf16 hi+lo and the product uses the 3-term expansion
      x.q ~= xh.qh + xh.ql + xl.qh   (drop lo*lo, ~8e-4 abs err;
                                      measured 2/81920 index mismatches)
    -- all-bf16 LDW+MM pairs stream at ~80 ns (FWL + reorder window).
  - Bias -||r||^2 via ONE bf16 K=3 matmul: ones3^T @ [b_hi;b_mid;b_lo]
    (3-way bf16 split is exact to fp32 precision).
  - All DMA on hardware DGE (one qSPDynamicHW queue, in-order): consts,
    then 5 x chunks with ramped sizes (2/2/4/4/4 row-tiles).
  - Top-5: DVE max (top-8 values desc) + max_index on the exact fp32
    distances (reduced-precision ranking keys measured too close to the
    2e-2 gate).  Ties get ascending indices, matching top_k.
  - Output: one [128, 16*8] u32 staging tile -> one dense [128,128] DRAM
    DMA (512B descriptors); the host de-interleaves tiles (free).
"""

import numpy as np
import ml_dtypes

import concourse.bass as bass  # noqa: F401  (AP helpers)
import concourse.mybir as mybir
from concourse import bacc, tile
from concourse.bass_utils import run_bass_kernel_spmd

N_CORES = 8
B = 16384          # total rows
D = 256            # feature dim
P = 100            # number of reference points
ROWS_PER_CORE = B // N_CORES      # 2048
ROW_TILE = 128
N_ROW_TILES = ROWS_PER_CORE // ROW_TILE   # 16
# chunk j issues alternately from SP (sync) and Activation queues so the
# HWDGE issue overhead (~650ns per dma_start) is paid on two queues in
# parallel; c0 goes first so the PE can start ASAP.
CHUNK_TILES = [1, 2, 3, 4, 6]

# bf16 consts layout (one [128, CONST_W] bf16 tensor):
#   [:, 0:100]       qh0   hi(2 r^T) rows 0..127
#   [:, 100:200]     ql0   lo(2 r^T) rows 0..127
#   [:, 200:300]     qh1   hi(2 r^T) rows 128..255
#   [:, 300:400]     ql1   lo(2 r^T) rows 128..255
#   [0:3, 400:528]   ones3 (K=3 lhsT for the bias matmul)
#   [0:3, 528:628]   bias hi/mid/lo rows of -||r||^2
CONST_W = 628

_cached = {}


def _build_bass():
    # Bacc (not plain Bass): its compile() runs move_matmul_waits_to_ldweights
    # + generate_event_semaphores, which split multi-sem waits to satisfy the
    # 1-wait-per-instruction hardware limit.
    nc = bacc.Bacc("TRN2")

    # xt[hl, a, p, n] = bf16 hi/lo part hl of x^T[a*128 + p, n]
    xt = nc.dram_tensor("xt", [2, 2, 128, ROWS_PER_CORE], mybir.dt.bfloat16,
                        kind="ExternalInput")
    consts = nc.dram_tensor("consts", [128, CONST_W], mybir.dt.bfloat16,
                            kind="ExternalInput")
    out_idx = nc.dram_tensor("out_idx", [128, N_ROW_TILES * 8],
                             mybir.dt.uint32, kind="ExternalOutput")

    xtv = xt.rearrange("h a p n -> p h a n")

    with tile.TileContext(nc) as tc:
        with (
            tc.tile_pool(name="consts", bufs=1) as cpool,
            tc.tile_pool(name="xt", bufs=1) as xpool,
            tc.tile_pool(name="v8", bufs=4) as vpool,
            tc.tile_pool(name="stage", bufs=1) as tpool,
            tc.tile_pool(name="psum", bufs=8, space="PSUM") as ppool,
        ):
            # consts on the Activation queue: its HWDGE generation runs in
            # parallel with c0's on the SP queue.
            consts_t = cpool.tile([128, CONST_W], mybir.dt.bfloat16)
            nc.scalar.dma_start(consts_t[:], consts[:])
            q_t = [[consts_t[:, 0:P], consts_t[:, P:2 * P]],          # half 0
                   [consts_t[:, 2 * P:3 * P], consts_t[:, 3 * P:4 * P]]]
            ones3_t = consts_t[0:3, 400:400 + ROW_TILE]
            bias3_t = consts_t[0:3, 528:528 + P]

            # Each HWDGE queue is in-order, so chunk completion sems fire at
            # ~proportional time, letting compute pipeline behind the stream.
            xt_t = []
            col = 0
            for j, ntiles in enumerate(CHUNK_TILES):
                w = ntiles * ROW_TILE
                t = xpool.tile([128, 2, 2, w], mybir.dt.bfloat16,
                               name=f"xt_{j}")
                eng = nc.sync if j % 2 == 0 else nc.scalar
                eng.dma_start(t[:], xtv[:, :, :, col:col + w])
                xt_t.append((t, col))
                col += w

            # all 16 row-tiles' index results accumulate here; one DMA out
            stage = tpool.tile([128, N_ROW_TILES * 8], mybir.dt.uint32,
                               name="stage", tag="stage")

            tile_chunk = []    # row-tile index -> (chunk tile, col offset)
            for (t, col), ntiles in zip(xt_t, CHUNK_TILES):
                for k in range(ntiles):
                    tile_chunk.append((t, k * ROW_TILE))

            for i in range(N_ROW_TILES):
                xt_tile, c = tile_chunk[i]
                p = ppool.tile([ROW_TILE, P], mybir.dt.float32,
                               name=f"psum_{i}", tag="psum")
                # PSUM = ones3^T @ (-||r||^2 as hi+mid+lo)
                nc.tensor.matmul(p[:], ones3_t, bias3_t,
                                 start=True, stop=False)
                # PSUM += xh.qh + xh.ql + xl.qh, both K-halves
                for a in (0, 1):
                    xh = xt_tile[:, 0, a, c:c + ROW_TILE]
                    xl = xt_tile[:, 1, a, c:c + ROW_TILE]
                    qh, ql = q_t[a]
                    nc.tensor.matmul(p[:], xh, qh, start=False, stop=False)
                    nc.tensor.matmul(p[:], xh, ql, start=False, stop=False)
                    nc.tensor.matmul(p[:], xl, qh, start=False,
                                     stop=(a == 1))

                # DVE reads the PSUM accumulator directly: no PSUM->SBUF
                # copy stage, one less cross-engine hop in the pipeline.
                v8 = vpool.tile([ROW_TILE, 8], mybir.dt.float32,
                                name=f"v8_{i}", tag="v8")
                nc.vector.max(out=v8[:], in_=p[:])
                nc.vector.max_index(out=stage[:, i * 8:(i + 1) * 8],
                                    in_max=v8[:], in_values=p[:])

            # dense [128, 128] store; host de-interleaves (tile, row) order
            nc.sync.dma_start(out_idx[:], stage[:])

    nc.compile()
    return nc


def _bf16_split(a32: np.ndarray):
    hi = a32.astype(ml_dtypes.bfloat16)
    lo = (a32 - hi.astype(np.float32)).astype(ml_dtypes.bfloat16)
    return hi, lo


def _make_consts(r: np.ndarray) -> np.ndarray:
    q = (2.0 * r.T.astype(np.float64)).astype(np.float32)      # [256, 100]
    b = (-(r.astype(np.float64) ** 2).sum(axis=1)).astype(np.float32)
    bh = b.astype(ml_dtypes.bfloat16)
    bm = (b - bh.astype(np.float32)).astype(ml_dtypes.bfloat16)
    bl = (b - bh.astype(np.float32)
          - bm.astype(np.float32)).astype(ml_dtypes.bfloat16)
    consts = np.zeros((128, CONST_W), dtype=ml_dtypes.bfloat16)
    for a in (0, 1):
        qh, ql = _bf16_split(q[a * 128:(a + 1) * 128])
        consts[:, 2 * a * P:(2 * a + 1) * P] = qh
        consts[:, (2 * a + 1) * P:(2 * a + 2) * P] = ql
    consts[0:3, 400:400 + ROW_TILE] = 1.0
    consts[0, 528:528 + P] = bh
    consts[1, 528:528 + P] = bm
    consts[2, 528:528 + P] = bl
    return consts


def kernel(x: np.ndarray, reference_points: np.ndarray) -> np.ndarray:
    assert x.shape == (B, D) and reference_points.shape == (P, D)
    x = np.asarray(x, dtype=np.float32)
    r = np.asarray(reference_points, dtype=np.float32)

    xt32 = np.ascontiguousarray(x.T)                    # [256, 16384]
    xh, xl = _bf16_split(xt32)
    # xt_all[hl, a, p, n] = part hl of x^T[a*128+p, n]
    xt_all = np.stack([xh.reshape(2, 128, B), xl.reshape(2, 128, B)])
    consts = _make_consts(r)

    if "nc" not in _cached:
        _cached["nc"] = _build_bass()
    nc = _cached["nc"]

    in_maps = []
    for c in range(N_CORES):
        slab = np.ascontiguousarray(
            xt_all[:, :, :, c * ROWS_PER_CORE:(c + 1) * ROWS_PER_CORE])
        in_maps.append({"xt": slab, "consts": consts})

    res = run_bass_kernel_spmd(nc, in_maps, core_ids=list(range(N_CORES)))
    _cached["last_result"] = res  # exec_time_ns etc. when BASS_TRACE=1

    # out_idx[p, t*8 + k] -> row t*128 + p, neighbor k
    outs = []
    for c in range(N_CORES):
        o = res.results[c]["out_idx"].reshape(128, N_ROW_TILES, 8)
        outs.append(o.transpose(1, 0, 2).reshape(ROWS_PER_CORE, 8)[:, :5])
    return np.concatenate(outs, axis=0).astype(np.int32)
